# revision 49
# baseline (speedup 1.0000x reference)
"""Trainium2 kernel for nn_Gamba (GIN message passing + attn-pool + mamba).

Single-launch fused implementation on 8 NeuronCores:
  - 3 gather-based GIN layers. Edge gathers use batched SWDGE dma_gather
    (1024 indices per instruction, round-robin over 4 SWDGE queues, int16
    indices => the node space is split into 4 chunks of 32768 rows).
  - Edge slots are packed densely per (tile-group, chunk) with compile-time
    per-(tile, chunk) capacities (max across cores so the SPMD program is
    shared); aggregation = one-hot selector matmuls into PSUM.
  - 2 attention-pool + mamba stages (tiny per-graph compute, SBUF resident).
  - 2 AllGather collectives (x after layers 0 and 1).
  - Final GIN layer is algebraically fused into the per-graph sum: each core
    computes per-graph partial sums of its local nodes with edge-count
    weights, applies W, then a tiny ReduceScatter yields each core's graphs.
"""
import sys
import time

sys.path.insert(0, '/opt/trn_rl_repo')


import numpy as np
import ml_dtypes

H = 128
T = 8
NH = 4
HD = H // NH
I = 256
S = 16
RK = 8
KC = 4
NCORES = 8
B = 128
GPC = B // NCORES  # 16
CH = 32768         # gather chunk rows (int16 index range)
GS = 4             # tiles per gather group
MAXIDX = 1024      # SWDGE ring: max descriptors per dma_gather
EVEN_PIECES = False

BF = ml_dtypes.bfloat16


def build_schedule(N):
    """Compile-time (cross-core shared) edge-slot schedule from capacities KT.

    Returns a dict of layout constants; KT is [TPC, NCH] per-(tile, chunk)
    slot capacity (max across cores).
    """
    NPC = N // NCORES
    TPC = NPC // 128
    NCH = (N + CH - 1) // CH
    NG = TPC // GS
    return dict(NPC=NPC, TPC=TPC, NCH=NCH, NG=NG)


def make_layout(KT, N):
    NPC = N // NCORES
    TPC = NPC // 128
    NCH = (N + CH - 1) // CH
    NG = TPC // GS

    O = np.zeros((TPC, NCH), np.int64)     # tile offset within (g, r) stream
    CP = np.zeros((NG, NCH), np.int64)     # padded (g, r) stream length
    for g in range(NG):
        for r in range(NCH):
            off = 0
            for t in range(g * GS, (g + 1) * GS):
                O[t, r] = off
                off += KT[t, r]
            CP[g, r] = ((off + 127) // 128) * 128

    IO = np.zeros((NG, NCH), np.int64)     # idx col offset (int16 cols)
    CO = np.zeros((NG, NCH), np.int64)     # gbuf H-col offset within group
    pieces = [[[] for _ in range(NCH)] for _ in range(NG)]
    io = 0
    for g in range(NG):
        co = 0
        for r in range(NCH):
            IO[g, r] = io
            CO[g, r] = co
            io += CP[g, r] // 16
            co += CP[g, r] // 128
            total = int(CP[g, r])
            if EVEN_PIECES:
                # equal-ish pieces (multiples of 128, <= MAXIDX)
                if total > 0:
                    np_ = -(-total // MAXIDX)          # ceil
                    base_sz = (total // np_) // 128 * 128
                    rem = total - base_sz * np_        # multiple of 128
                    pos = 0
                    for pi in range(np_):
                        nn_ = base_sz + (128 if pi < rem // 128 else 0)
                        pieces[g][r].append((int(pos), int(nn_)))
                        pos += nn_
                    assert pos == total
            else:
                pos = 0
                while pos < total:
                    nn_ = min(MAXIDX, total - pos)
                    pieces[g][r].append((int(pos), int(nn_)))
                    pos += nn_
    TOTI16 = int(io)
    GW = [int(CP[g].sum() // 128) for g in range(NG)]
    WMAX = max(GW)

    jlo = np.zeros((TPC, NCH), np.int64)
    jhi = np.zeros((TPC, NCH), np.int64)
    for t in range(TPC):
        for r in range(NCH):
            b = O[t, r]
            e = b + KT[t, r]
            jlo[t, r] = b // 128
            jhi[t, r] = (e + 127) // 128 if e > b else b // 128
    ncols = (jhi - jlo).sum(axis=1)        # selector cols per tile
    DLB = np.zeros(TPC + 1, np.int64)
    DLB[1:] = np.cumsum(ncols)
    TOTD = int(DLB[-1])
    # rcb[t, r]: dl col base for (t, r)
    rcb = np.zeros((TPC, NCH), np.int64)
    for t in range(TPC):
        acc = DLB[t]
        for r in range(NCH):
            rcb[t, r] = acc
            acc += jhi[t, r] - jlo[t, r]
    return dict(O=O, CP=CP, IO=IO, CO=CO, pieces=pieces, TOTI16=TOTI16,
                WMAX=WMAX, jlo=jlo, jhi=jhi, ncols=ncols, DLB=DLB,
                TOTD=TOTD, rcb=rcb, NG=NG, TPC=TPC, NCH=NCH)


def prep_edges(edge_index, N, NPG):
    """Pack edges (+self loops) into per-core idx (int16, Q7-replicated) and
    dl (selector target rows) arrays, plus per-core cnt2 [NPC, B] weights for
    the fused layer-3 local graph-sum."""
    NPC = N // NCORES
    TPC = NPC // 128
    NCH = (N + CH - 1) // CH

    src = edge_index[0].astype(np.int64)
    dst = edge_index[1].astype(np.int64)

    # fused final layer: per-core cnt2[n_local, G] = #edges (c-local n -> G)
    # + self-loop membership
    G_of = dst // NPG
    cnt2_core = []
    for c in range(NCORES):
        cnt2 = np.zeros((NPC, B), np.float32)
        m = (src >= c * NPC) & (src < (c + 1) * NPC)
        np.add.at(cnt2, (src[m] - c * NPC, G_of[m]), 1.0)
        own = np.arange(NPC)
        cnt2[own, (own + c * NPC) // NPG] += 1.0
        cnt2_core.append(cnt2.astype(BF))

    # self-loops are not packed: the self term is injected on-device via an
    # identity matmul from the SBUF-resident natural-orientation x.
    core = dst // NPC
    tl = (dst % NPC) // 128
    drow = (dst % 128).astype(np.int64)
    ch = src // CH
    v16 = (src % CH).astype(np.int16)

    cnt = np.zeros((NCORES, TPC, NCH), np.int64)
    np.add.at(cnt, (core, tl, ch), 1)
    KT = cnt.max(axis=0)

    lay = make_layout(KT, N)
    O, IO, jlo, rcb = lay['O'], lay['IO'], lay['jlo'], lay['rcb']
    TOTI16, TOTD = lay['TOTI16'], lay['TOTD']

    idx_core, dl_core = [], []
    for c in range(NCORES):
        m = core == c
        tl_c, ch_c, v_c, dr_c = tl[m], ch[m], v16[m], drow[m]
        key = tl_c * NCH + ch_c
        order = np.argsort(key, kind='stable')
        tl_s, ch_s, v_s, dr_s = tl_c[order], ch_c[order], v_c[order], dr_c[order]
        ks = key[order]
        cnts = np.bincount(ks, minlength=TPC * NCH)
        gstart = np.zeros(TPC * NCH, np.int64)
        gstart[1:] = np.cumsum(cnts)[:-1]
        within = np.arange(len(ks)) - gstart[ks]
        g_s = tl_s // GS
        pos = O[tl_s, ch_s] + within
        col16 = IO[g_s, ch_s] + pos // 16
        row16 = pos % 16
        # Padding entries gather garbage that the selector zeroes out, but
        # their DMA still runs: identical values (e.g. all 0) serialize on
        # one HBM row (measured ~9 GB/s); spread them across the chunk.
        prng = np.random.default_rng(12345 + c)
        blk = prng.integers(0, CH, size=(16, TOTI16)).astype(np.int16)
        blk[row16, col16] = v_s
        idx_core.append(np.tile(blk, (8, 1)))

        dlc = np.full((128, TOTD), -1.0, np.float32)
        dcol = rcb[tl_s, ch_s] + (pos // 128 - jlo[tl_s, ch_s])
        dlc[pos % 128, dcol] = dr_s.astype(np.float32)
        dl_core.append(dlc.astype(BF))
    return lay, idx_core, dl_core, cnt2_core


def build_fused(N, NPG, lay, R=1, abl='full'):
    import concourse.bass as bass
    from concourse import bacc
    import concourse.mybir as mybir
    import concourse.tile as tile

    f32, bf16 = mybir.dt.float32, mybir.dt.bfloat16
    i16 = mybir.dt.int16
    AF = mybir.ActivationFunctionType
    OP = mybir.AluOpType

    NPC = N // NCORES
    TPC = NPC // 128
    TPG = NPG // 128
    NCH, NG = lay['NCH'], lay['NG']
    CO, IO, pieces = lay['CO'], lay['IO'], lay['pieces']
    jlo, jhi, ncols, DLB = lay['jlo'], lay['jhi'], lay['ncols'], lay['DLB']
    TOTI16, TOTD, WMAX = lay['TOTI16'], lay['TOTD'], lay['WMAX']
    NCMAX = int(ncols.max())
    assert TPG * 128 == NPG and GPC * TPG == TPC

    nc = bacc.Bacc('TRN2', num_devices=NCORES, num_swdge_queues=4)

    def din(name, shape, dt):
        return nc.dram_tensor(name, shape, dt, kind='ExternalInput')

    xf = din('xf', [N, H], bf16)
    xown = din('xown', [NPC, H], bf16)
    ix16 = din('ix16', [128, TOTI16], i16)
    dl = din('dl', [128, TOTD], bf16)
    cnt2 = din('cnt2', [NPC, B], bf16)
    iota = din('iota', [128, 128], bf16)
    id128 = din('id128', [128, 128], bf16)
    id32 = din('id32', [32, 32], bf16)
    ones1 = din('ones1', [1, 128], bf16)
    selb = din('selb', [GPC, GPC * 128], bf16)
    wg = din('wg', [4, H, H], bf16)
    binrow = din('binrow', [1, H], bf16)
    bcols = din('bcols', [128, 4], f32)
    npgbrep = din('npgbrep', [GPC, 128], f32)
    qp = din('qp', [2, 128, 32], bf16)
    sbias = din('sbias', [2, 32, 1], f32)
    wv = din('wv', [2, 128, 128], bf16)
    aow = din('aow', [2, 128, 128], bf16)
    aob2 = din('aob2', [2, 1, 128], bf16)
    inw = din('inw', [128, 2 * I], bf16)
    cwm = din('cwm', [KC, 128, I], f32)
    cbrep = din('cbrep', [128, I], f32)
    xw = din('xw', [2, 128, RK + 2 * S], bf16)
    dtw = din('dtw', [RK, I], bf16)
    dtbrow = din('dtbrow', [1, I], bf16)
    arep = din('arep', [128, I * S], f32)
    drep = din('drep', [128, I], f32)
    nwrep = din('nwrep', [128, H], f32)
    nfwrep = din('nfwrep', [128, H], f32)
    shm = din('shm', [128, 5], f32)
    ivm = din('ivm', [128, 5], f32)
    outw = din('outw', [2, 128, H], bf16)
    epscol = din('epscol', [128, 1], f32)

    yout = nc.dram_tensor('yout', [GPC, 128], f32, kind='ExternalOutput')

    with tile.TileContext(nc) as tc:
        with tc.tile_pool(name='const', bufs=1) as cpool, \
             tc.tile_pool(name='dramp', bufs=1, space='DRAM') as dpool, \
             tc.tile_pool(name='ixp', bufs=3) as ixp, \
             tc.tile_pool(name='spool', bufs=3) as spool, \
             tc.tile_pool(name='cntp', bufs=3) as cntp:

            # ---- constants into SBUF ----
            def sb_const(t_dram, shape, dt):
                tl_ = cpool.tile(shape, dt, name=f'c_{t_dram.name}')
                nc.sync.dma_start(out=tl_[:],
                                  in_=t_dram[(slice(None),) * len(shape)])
                return tl_

            io_sb = sb_const(iota, [128, 128], bf16)
            id128_sb = sb_const(id128, [128, 128], bf16)
            id32_sb = sb_const(id32, [32, 32], bf16)
            ones1_sb = sb_const(ones1, [1, 128], bf16)
            selb_sb = sb_const(selb, [GPC, GPC * 128], bf16)
            dl_sb = sb_const(dl, [128, TOTD], bf16)
            binrow_sb = sb_const(binrow, [1, H], bf16)
            bcols_sb = sb_const(bcols, [128, 4], f32)
            npgb_sb = sb_const(npgbrep, [GPC, 128], f32)
            inw_sb = sb_const(inw, [128, 2 * I], bf16)
            cbrep_sb = sb_const(cbrep, [128, I], f32)
            dtw_sb = sb_const(dtw, [RK, I], bf16)
            dtbrow_sb = sb_const(dtbrow, [1, I], bf16)
            arep_sb = sb_const(arep, [128, I * S], f32)
            drep_sb = sb_const(drep, [128, I], f32)
            nwrep_sb = sb_const(nwrep, [128, H], f32)
            nfwrep_sb = sb_const(nfwrep, [128, H], f32)
            shm_sb = sb_const(shm, [128, 5], f32)
            ivm_sb = sb_const(ivm, [128, 5], f32)
            eps_sb = sb_const(epscol, [128, 1], f32)

            wg_sb, qp_sb, sbias_sb = [], [], []
            wv_sb, aow_sb, aob2_sb, cwm_sb, xw_sb = [], [], [], [], []
            for l in range(4):
                tl_ = cpool.tile([H, H], bf16, name=f'c_wg{l}')
                nc.sync.dma_start(out=tl_[:], in_=wg[l, :, :])
                wg_sb.append(tl_)
            for l in range(2):
                for lst, src_, shape, dt in (
                        (qp_sb, qp, [128, 32], bf16),
                        (sbias_sb, sbias, [32, 1], f32),
                        (wv_sb, wv, [128, 128], bf16),
                        (aow_sb, aow, [128, 128], bf16),
                        (aob2_sb, aob2, [1, 128], bf16),
                        (xw_sb, xw, [128, RK + 2 * S], bf16)):
                    tl_ = cpool.tile(shape, dt, name=f'c_{src_.name}{l}')
                    nc.sync.dma_start(out=tl_[:], in_=src_[l, :, :])
                    lst.append(tl_)
            for k in range(KC):
                tl_ = cpool.tile([128, I], f32, name=f'c_cwm{k}')
                nc.sync.dma_start(out=tl_[:], in_=cwm[k, :, :])
                cwm_sb.append(tl_)
            outw_sb = []
            for c in range(2):
                tl_ = cpool.tile([128, H], bf16, name=f'c_outw{c}')
                nc.sync.dma_start(out=tl_[:], in_=outw[c, :, :])
                outw_sb.append(tl_)

            # ---- persistent SBUF state ----
            xT_sb = cpool.tile([128, NPC], bf16, name='xT_sb')
            xN_sb = cpool.tile([128, NPC], bf16, name='xN_sb')
            aggall_sb = cpool.tile([128, NPC], bf16, name='aggall_sb')
            st_sb = cpool.tile([128, B], f32, name='st_sb')
            tokens_sb = cpool.tile([128, H], f32, name='tokens_sb')
            tokT_sb = cpool.tile([128, GPC * T], bf16, name='tokT_sb')
            grow_sb = [cpool.tile([GPC, 128], bf16, name=f'grow{l}')
                       for l in range(2)]
            bgcol_sb = [cpool.tile([128, GPC], f32, name=f'bgcol{l}')
                        for l in range(2)]
            if abl != 'full':
                for l in range(2):
                    nc.vector.memset(grow_sb[l][:], 0.0)
                    nc.vector.memset(bgcol_sb[l][:], 0.0)

            uid = [0]
            qctr = [0]

            # =========== GIN building blocks ===========
            GIW = [int(sum(lay['CP'][g]) // 16) for g in range(NG)]
            GIWMAX = max(GIW)

            def emit_gathers(g, src_dram, gpool):
                ib = int(IO[g, 0])
                ixg = ixp.tile([128, GIWMAX], i16, tag='ixg')
                nc.sync.dma_start(out=ixg[:, 0:GIW[g]],
                                  in_=ix16[:, ib:ib + GIW[g]])
                g_b = gpool.tile([128, WMAX * H], bf16, tag='g_b')
                if abl == 'nogather':
                    nc.vector.memset(g_b[:], 0.0)
                    return g_b
                for r in range(NCH):
                    for (pos0, nn_) in pieces[g][r]:
                        c0 = int(CO[g, r]) + pos0 // 128
                        i0 = int(IO[g, r]) - ib + pos0 // 16
                        nc.gpsimd.dma_gather(
                            out_ap=g_b[:, c0 * H:(c0 + nn_ // 128) * H]
                                .rearrange('p (c h) -> p c h', h=H),
                            in_ap=src_dram[r * CH:(r + 1) * CH, :],
                            idxs_ap=ixg[:, i0:i0 + nn_ // 16],
                            num_idxs=nn_,
                            num_idxs_reg=nn_,
                            elem_size=H,
                            queue_num=qctr[0] % 4,
                            single_packet=False,
                        )
                        qctr[0] += 1
                return g_b

            def emit_agg(g, t, g_b, pps, out_sl):
                """selector build + aggregation matmuls; agg^T -> out_sl."""
                nct = int(ncols[t])
                dlb = int(DLB[t])
                cols = []
                for r in range(NCH):
                    for j in range(int(jlo[t, r]), int(jhi[t, r])):
                        cols.append(int(CO[g, r]) + j)
                assert len(cols) == nct and nct > 0
                s_b = spool.tile([128, NCMAX * 128], bf16, tag='s_b')
                nc.vector.tensor_tensor(
                    out=s_b[:, 0:nct * 128]
                        .rearrange('p (j d) -> p j d', j=nct),
                    in0=dl_sb[:, dlb:dlb + nct]
                        .rearrange('p (j o) -> p j o', o=1)
                        .to_broadcast([128, nct, 128]),
                    in1=io_sb[:].rearrange('p (o d) -> p o d', o=1)
                        .to_broadcast([128, nct, 128]),
                    op=OP.is_equal,
                )
                aggps = pps.tile([128, 128], f32, space='PSUM', tag='aggps')
                # self term: agg^T[h, c] += x[c, h] via identity
                nc.tensor.matmul(
                    out=aggps[:],
                    lhsT=xN_sb[:, t * 128:(t + 1) * 128],
                    rhs=id128_sb[:],
                    start=True, stop=False,
                )
                for k, col in enumerate(cols):
                    nc.tensor.matmul(
                        out=aggps[:],
                        lhsT=g_b[:, col * H:(col + 1) * H],
                        rhs=s_b[:, k * 128:(k + 1) * 128],
                        start=False, stop=(k == nct - 1),
                    )
                nc.scalar.copy(out=out_sl, in_=aggps[:])

            def emit_y(l, t, agg_sl, pps, xloc, st_pps):
                """y = agg^T W + bias; writes xloc/xT (l<2) or gsum (l==2)."""
                gg = t // TPG
                yps = pps.tile([128, 128], f32, space='PSUM', tag='ynt')
                nc.tensor.matmul(out=yps[:], lhsT=agg_sl,
                                 rhs=wg_sb[l][:], start=True, stop=False)
                if l == 0:
                    nc.tensor.matmul(out=yps[:], lhsT=ones1_sb[0:1, :],
                                     rhs=binrow_sb[0:1, :],
                                     start=False, stop=True)
                else:
                    nc.tensor.matmul(
                        out=yps[:],
                        lhsT=selb_sb[:, gg * 128:(gg + 1) * 128],
                        rhs=grow_sb[l - 1][:],
                        start=False, stop=True)
                y_sb = xN_sb[:, t * 128:(t + 1) * 128]
                nc.scalar.copy(out=y_sb, in_=yps[:])
                if l < 2:
                    nc.sync.dma_start(
                        out=xloc[l][t * 128:(t + 1) * 128, :], in_=y_sb)
                    ytps = pps.tile([128, 128], f32, space='PSUM', tag='ynt')
                    nc.tensor.matmul(out=ytps[:], lhsT=wg_sb[l][:],
                                     rhs=agg_sl, start=True, stop=True)
                    bias = (bcols_sb[:, 0:1] if l == 0
                            else bgcol_sb[l - 1][:, gg:gg + 1])
                    nc.scalar.activation(
                        out=xT_sb[:, t * 128:(t + 1) * 128],
                        in_=ytps[:], func=AF.Identity, bias=bias)
                else:
                    cnt_t = cntp.tile([128, B], bf16, tag='cnt_t')
                    nc.sync.dma_start(out=cnt_t[:],
                                      in_=cnt2[t * 128:(t + 1) * 128, :])
                    stp_ps = pps.tile([128, B], f32, space='PSUM', tag='ynt')
                    nc.tensor.matmul(out=stp_ps[:], lhsT=y_sb,
                                     rhs=cnt_t[:], start=True, stop=True)
                    nc.vector.tensor_tensor(out=st_sb[:], in0=st_sb[:],
                                            in1=stp_ps[:], op=OP.add)

            def gin0(xloc):
                """Layer 0: single fused pass (bias is a constant row)."""
                uid[0] += 1
                u = uid[0]
                with tc.tile_pool(name=f'ps0{u}', bufs=2,
                                  space='PSUM') as pps, \
                     tc.tile_pool(name=f'gp0{u}', bufs=2) as gpool, \
                     tc.tile_pool(name=f'ag0{u}', bufs=3) as apool:
                    for g in range(NG):
                        g_b = emit_gathers(g, xf, gpool)
                        if abl == 'gonly':
                            continue
                        for t in range(g * GS, (g + 1) * GS):
                            aggsb = apool.tile([128, 128], bf16, tag='agg')
                            emit_agg(g, t, g_b, pps, aggsb[:])
                            emit_y(0, t, aggsb[:], pps, xloc, None)

            def ginAB(l, src_dram, xloc, st_pps, sl):
                """Layers 1/2: pass A (gather+agg, stage graphs interleaved),
                then mamba of stage sl, then pass B (weights+bias)."""
                uid[0] += 1
                u = uid[0]
                do_st = abl == 'full'
                with tc.tile_pool(name=f'psA{u}', bufs=2,
                                  space='PSUM') as pps, \
                     tc.tile_pool(name=f'gpA{u}', bufs=2) as gpool, \
                     tc.tile_pool(name=f'at{u}', bufs=2) as atp, \
                     tc.tile_pool(name=f'atps{u}', bufs=2,
                                  space='PSUM') as atps:
                    for g in range(NG):
                        g_b = emit_gathers(g, src_dram, gpool)
                        if do_st and g % 2 == 1:
                            stage_graph(sl, g // 2, atp, atps)
                        if abl == 'gonly':
                            continue
                        for t in range(g * GS, (g + 1) * GS):
                            emit_agg(g, t, g_b, pps,
                                     aggall_sb[:, t * 128:(t + 1) * 128])
                if do_st:
                    stage_mamba(sl)
                if abl == 'gonly':
                    return
                with tc.tile_pool(name=f'psB{u}', bufs=2,
                                  space='PSUM') as ppsB:
                    for t in range(TPC):
                        emit_y(l, t, aggall_sb[:, t * 128:(t + 1) * 128],
                               ppsB, xloc, st_pps)

            # =========== attention + mamba stage ===========
            def stage_graph(l, g, atp, atps):
                """Attention pool for one graph: V precompute, scores with
                Wk folded into qp, softmax, batched e-transposes, o^T."""
                csz = min(512, NPG)
                NCK = NPG // 128
                # V chunks first: independent of the softmax chain
                vnmall = atp.tile([128, NCK * 128], bf16, tag='vnmall')
                for c in range(NCK):
                    vps = atps.tile([128, 128], f32, space='PSUM',
                                    tag='vps', bufs=2)
                    nc.tensor.matmul(
                        out=vps[:],
                        lhsT=xT_sb[:, g * NPG + c * 128:
                                   g * NPG + (c + 1) * 128],
                        rhs=wv_sb[l][:], start=True, stop=True)
                    nc.scalar.copy(out=vnmall[:, c * 128:(c + 1) * 128],
                                   in_=vps[:])
                sc = atp.tile([32, NPG], f32, tag='sc', bufs=1)
                for c in range(NPG // csz):
                    scps = atps.tile([128, csz], f32, space='PSUM',
                                     tag='big', bufs=1)
                    nc.tensor.matmul(
                        out=scps[0:32, :], lhsT=qp_sb[l][:],
                        rhs=xT_sb[:, g * NPG + c * csz:
                                  g * NPG + (c + 1) * csz],
                        start=True, stop=True)
                    nc.scalar.activation(
                        out=sc[:, c * csz:(c + 1) * csz],
                        in_=scps[0:32, :], func=AF.Identity,
                        bias=sbias_sb[l][:, 0:1])
                mxn = atp.tile([32, 1], f32, tag='mxn')
                nc.vector.tensor_reduce(
                    out=mxn[:], in_=sc[:],
                    axis=mybir.AxisListType.X, op=OP.max, negate=True)
                e_sb = atp.tile([32, NPG], bf16, tag='e_sb', bufs=1)
                nc.scalar.activation(out=e_sb[:], in_=sc[:],
                                     func=AF.Exp, bias=mxn[:, 0:1])
                sm = atp.tile([32, 1], f32, tag='sm')
                nc.vector.tensor_reduce(
                    out=sm[:], in_=e_sb[:],
                    axis=mybir.AxisListType.X, op=OP.add)
                rq = atp.tile([32, 1], f32, tag='rq')
                nc.vector.reciprocal(out=rq[:], in_=sm[:])
                nc.vector.tensor_scalar(
                    out=e_sb[:], in0=e_sb[:], scalar1=rq[:, 0:1],
                    scalar2=None, op0=OP.mult)
                # all e-chunk transposes into one PSUM tile, one copy out
                tpsall = atps.tile([128, NCK * 32], bf16, space='PSUM',
                                   tag='tps', bufs=1)
                for c in range(NCK):
                    nc.tensor.transpose(
                        tpsall[:, c * 32:(c + 1) * 32],
                        e_sb[:, c * 128:(c + 1) * 128],
                        id32_sb[:])
                atnall = atp.tile([128, NCK * 32], bf16, tag='atn')
                nc.scalar.copy(out=atnall[:], in_=tpsall[:])
                oT_ps = atps.tile([128, 32], f32, space='PSUM',
                                  tag='oT', bufs=1)
                for c in range(NCK):
                    nc.tensor.matmul(
                        out=oT_ps[:],
                        lhsT=vnmall[:, c * 128:(c + 1) * 128],
                        rhs=atnall[:, c * 32:(c + 1) * 32],
                        start=(c == 0), stop=(c == NCK - 1))
                tks = atp.tile([128, T], bf16, tag='tks')
                for hh in range(NH):
                    nc.scalar.copy(
                        out=tks[hh * HD:(hh + 1) * HD, :],
                        in_=oT_ps[hh * HD:(hh + 1) * HD,
                                  hh * T:(hh + 1) * T])
                tokT_ps = atps.tile([128, T], f32, space='PSUM',
                                    tag='tokT', bufs=1)
                nc.tensor.matmul(out=tokT_ps[:], lhsT=aow_sb[l][:],
                                 rhs=tks[:], start=True, stop=False)
                nc.tensor.matmul(out=tokT_ps[:],
                                 lhsT=aob2_sb[l][0:1, :],
                                 rhs=ones1_sb[0:1, 0:T],
                                 start=False, stop=True)
                nc.scalar.copy(out=tokT_sb[:, g * T:(g + 1) * T],
                               in_=tokT_ps[:])

            def stage_mamba(l):
                lcol = l + 1
                # ---- mamba on tokens [128 (g,t), H] ----
                with tc.tile_pool(name=f'mm{l}u{uid[0]}', bufs=1) as mp, \
                     tc.tile_pool(name=f'scn{l}u{uid[0]}', bufs=1) as scanp, \
                     tc.tile_pool(name=f'mps{l}u{uid[0]}', bufs=1,
                                  space='PSUM') as mps:
                    tk_ps = mps.tile([128, 128], bf16, space='PSUM',
                                     tag='tr', bufs=1)
                    nc.tensor.transpose(tk_ps[:], tokT_sb[:], id128_sb[:])
                    nc.scalar.copy(out=tokens_sb[:], in_=tk_ps[:])
                    scr = mp.tile([128, H], f32, name='scr')
                    ms = mp.tile([128, 1], f32, name='ms')
                    nc.scalar.activation(out=scr[:], in_=tokens_sb[:],
                                         func=AF.Square, accum_out=ms[:])
                    srt = mp.tile([128, 1], f32, name='srt')
                    nc.scalar.activation(out=srt[:], in_=ms[:], func=AF.Sqrt,
                                         scale=1.0 / H, bias=eps_sb[:, 0:1])
                    rs = mp.tile([128, 1], f32, name='rs')
                    nc.vector.reciprocal(out=rs[:], in_=srt[:])
                    hn = mp.tile([128, H], f32, name='hn')
                    nc.vector.tensor_scalar(out=hn[:], in0=tokens_sb[:],
                                            scalar1=rs[:, 0:1], scalar2=None,
                                            op0=OP.mult)
                    nc.vector.tensor_tensor(out=hn[:], in0=hn[:],
                                            in1=nwrep_sb[:], op=OP.mult)
                    hnb = mp.tile([128, H], bf16, name='hnb')
                    nc.vector.tensor_copy(out=hnb[:], in_=hn[:])
                    hT_ps = mps.tile([128, 128], bf16, space='PSUM',
                                     tag='tr', bufs=1)
                    nc.tensor.transpose(hT_ps[:], hnb[:], id128_sb[:])
                    hTb = mp.tile([128, H], bf16, name='hTb')
                    nc.scalar.copy(out=hTb[:], in_=hT_ps[:])
                    proj_ps = mps.tile([128, 2 * I], f32, space='PSUM',
                                       tag='proj')
                    nc.tensor.matmul(out=proj_ps[:], lhsT=hTb[:],
                                     rhs=inw_sb[:], start=True, stop=True)
                    u = mp.tile([128, I], f32, name='u')
                    nc.scalar.copy(out=u[:], in_=proj_ps[:, 0:I])
                    gate = mp.tile([128, I], f32, name='gate')
                    nc.scalar.activation(out=gate[:], in_=proj_ps[:, I:2 * I],
                                         func=AF.Silu)
                    cv = mp.tile([128, I], f32, name='cv')
                    nc.vector.tensor_tensor(out=cv[:], in0=u[:],
                                            in1=cwm_sb[0][:], op=OP.mult)
                    for jj in range(1, KC):
                        ush = mp.tile([128, I], f32, name=f'ush{jj}')
                        nc.vector.memset(ush[0:jj, :], 0.0)
                        nc.sync.dma_start(out=ush[jj:128, :],
                                          in_=u[0:128 - jj, :])
                        nc.vector.tensor_tensor(out=ush[:], in0=ush[:],
                                                in1=cwm_sb[jj][:],
                                                op=OP.mult)
                        nc.vector.tensor_tensor(out=cv[:], in0=cv[:],
                                                in1=ush[:], op=OP.add)
                    nc.vector.tensor_tensor(out=cv[:], in0=cv[:],
                                            in1=cbrep_sb[:], op=OP.add)
                    u2 = mp.tile([128, I], f32, name='u2')
                    nc.scalar.activation(out=u2[:], in_=cv[:], func=AF.Silu)
                    u2b = mp.tile([128, I], bf16, name='u2b')
                    nc.vector.tensor_copy(out=u2b[:], in_=u2[:])
                    uTb = []
                    for c in range(2):
                        trp = mps.tile([128, 128], bf16, space='PSUM',
                                       tag='tr', bufs=1)
                        nc.tensor.transpose(
                            trp[:], u2b[:, c * 128:(c + 1) * 128],
                            id128_sb[:])
                        ub = mp.tile([128, 128], bf16, name=f'uTb{c}')
                        nc.scalar.copy(out=ub[:], in_=trp[:])
                        uTb.append(ub)
                    ssm_ps = mps.tile([128, RK + 2 * S], f32, space='PSUM',
                                      tag='ssm')
                    nc.tensor.matmul(out=ssm_ps[:], lhsT=uTb[0][:],
                                     rhs=xw_sb[0][:], start=True, stop=False)
                    nc.tensor.matmul(out=ssm_ps[:], lhsT=uTb[1][:],
                                     rhs=xw_sb[1][:], start=False, stop=True)
                    ssmb = mp.tile([128, RK + 2 * S], bf16, name='ssmb')
                    nc.scalar.copy(out=ssmb[:], in_=ssm_ps[:])
                    Bm = mp.tile([128, S], f32, name='Bm')
                    nc.scalar.copy(out=Bm[:], in_=ssm_ps[:, RK:RK + S])
                    Cm = mp.tile([128, S], f32, name='Cm')
                    nc.scalar.copy(out=Cm[:],
                                   in_=ssm_ps[:, RK + S:RK + 2 * S])
                    dtr_ps = mps.tile([RK, 128], bf16, space='PSUM',
                                      tag='dtr')
                    nc.tensor.transpose(dtr_ps[:], ssmb[:, 0:RK],
                                        id128_sb[:])
                    dtrb = mp.tile([RK, 128], bf16, name='dtrb')
                    nc.scalar.copy(out=dtrb[:], in_=dtr_ps[:])
                    dt_ps = mps.tile([128, I], f32, space='PSUM', tag='dtp')
                    nc.tensor.matmul(out=dt_ps[:], lhsT=dtrb[:],
                                     rhs=dtw_sb[:], start=True, stop=False)
                    nc.tensor.matmul(out=dt_ps[:], lhsT=ones1_sb[0:1, :],
                                     rhs=dtbrow_sb[0:1, :], start=False,
                                     stop=True)
                    dte = mp.tile([128, I], f32, name='dte')
                    nc.scalar.activation(out=dte[:], in_=dt_ps[:],
                                         func=AF.Exp)
                    dt = mp.tile([128, I], f32, name='dt')
                    nc.scalar.activation(out=dt[:], in_=dte[:],
                                         func=AF.Ln, bias=1.0)
                    y = mp.tile([128, I], f32, name='y')
                    IH = I // 4
                    for hf in range(4):
                        i0 = hf * IH
                        dA = scanp.tile([128, IH * S], f32, tag='dA')
                        nc.vector.tensor_tensor(
                            out=dA[:].rearrange('p (i s) -> p i s', s=S),
                            in0=dt[:, i0:i0 + IH]
                                .rearrange('p (i o) -> p i o', o=1)
                                .to_broadcast([128, IH, S]),
                            in1=arep_sb[:, i0 * S:(i0 + IH) * S]
                                .rearrange('p (i s) -> p i s', s=S),
                            op=OP.mult)
                        nc.scalar.activation(out=dA[:], in_=dA[:],
                                             func=AF.Exp)
                        du = mp.tile([128, IH], f32, tag='du')
                        nc.vector.tensor_tensor(out=du[:],
                                                in0=dt[:, i0:i0 + IH],
                                                in1=u2[:, i0:i0 + IH],
                                                op=OP.mult)
                        dBu = scanp.tile([128, IH * S], f32, tag='dBu')
                        nc.vector.tensor_tensor(
                            out=dBu[:].rearrange('p (i s) -> p i s', s=S),
                            in0=du[:].rearrange('p (i o) -> p i o', o=1)
                                .to_broadcast([128, IH, S]),
                            in1=Bm[:].rearrange('p (o s) -> p o s', o=1)
                                .to_broadcast([128, IH, S]),
                            op=OP.mult)
                        ash = scanp.tile([128, IH * S], f32, tag='ash')
                        bsh = scanp.tile([128, IH * S], f32, tag='bsh')
                        for sh in (1, 2, 4):
                            nc.vector.memset(ash[0:sh, :], 1.0)
                            nc.vector.memset(bsh[0:sh, :], 0.0)
                            nc.sync.dma_start(out=ash[sh:128, :],
                                              in_=dA[0:128 - sh, :])
                            nc.sync.dma_start(out=bsh[sh:128, :],
                                              in_=dBu[0:128 - sh, :])
                            nc.vector.tensor_scalar(
                                out=ash[:], in0=ash[:],
                                scalar1=shm_sb[:, sh:sh + 1],
                                scalar2=ivm_sb[:, sh:sh + 1],
                                op0=OP.mult, op1=OP.add)
                            nc.vector.tensor_scalar(
                                out=bsh[:], in0=bsh[:],
                                scalar1=shm_sb[:, sh:sh + 1],
                                scalar2=None, op0=OP.mult)
                            nc.vector.tensor_tensor(out=bsh[:], in0=dA[:],
                                                    in1=bsh[:], op=OP.mult)
                            nc.vector.tensor_tensor(out=dBu[:], in0=dBu[:],
                                                    in1=bsh[:], op=OP.add)
                            nc.vector.tensor_tensor(out=dA[:], in0=dA[:],
                                                    in1=ash[:], op=OP.mult)
                        nc.vector.tensor_tensor(
                            out=ash[:].rearrange('p (i s) -> p i s', s=S),
                            in0=dBu[:].rearrange('p (i s) -> p i s', s=S),
                            in1=Cm[:].rearrange('p (o s) -> p o s', o=1)
                                .to_broadcast([128, IH, S]),
                            op=OP.mult)
                        nc.vector.tensor_reduce(
                            out=y[:, i0:i0 + IH],
                            in_=ash[:].rearrange('p (i s) -> p i s', s=S),
                            axis=mybir.AxisListType.X, op=OP.add)
                    nc.vector.tensor_tensor(out=u2[:], in0=u2[:],
                                            in1=drep_sb[:], op=OP.mult)
                    nc.vector.tensor_tensor(out=y[:], in0=y[:], in1=u2[:],
                                            op=OP.add)
                    nc.vector.tensor_tensor(out=y[:], in0=y[:], in1=gate[:],
                                            op=OP.mult)
                    yb = mp.tile([128, I], bf16, name='yb')
                    nc.vector.tensor_copy(out=yb[:], in_=y[:])
                    yTb = []
                    for c in range(2):
                        trp = mps.tile([128, 128], bf16, space='PSUM',
                                       tag='tr', bufs=1)
                        nc.tensor.transpose(
                            trp[:], yb[:, c * 128:(c + 1) * 128],
                            id128_sb[:])
                        ob = mp.tile([128, 128], bf16, name=f'yTb{c}')
                        nc.scalar.copy(out=ob[:], in_=trp[:])
                        yTb.append(ob)
                    gf_ps = mps.tile([128, H], f32, space='PSUM', tag='gf')
                    nc.tensor.matmul(out=gf_ps[:], lhsT=yTb[0][:],
                                     rhs=outw_sb[0][:], start=True,
                                     stop=False)
                    nc.tensor.matmul(out=gf_ps[:], lhsT=yTb[1][:],
                                     rhs=outw_sb[1][:], start=False,
                                     stop=True)
                    gf = mp.tile([128, H], f32, name='gf')
                    nc.scalar.copy(out=gf[:], in_=gf_ps[:])
                    nc.vector.tensor_tensor(out=gf[:], in0=gf[:],
                                            in1=tokens_sb[:], op=OP.add)
                    nc.scalar.activation(out=scr[:], in_=gf[:],
                                         func=AF.Square, accum_out=ms[:])
                    nc.scalar.activation(out=srt[:], in_=ms[:], func=AF.Sqrt,
                                         scale=1.0 / H, bias=eps_sb[:, 0:1])
                    nc.vector.reciprocal(out=rs[:], in_=srt[:])
                    nc.vector.tensor_scalar(out=gf[:], in0=gf[:],
                                            scalar1=rs[:, 0:1], scalar2=None,
                                            op0=OP.mult)
                    nc.vector.tensor_tensor(out=gf[:], in0=gf[:],
                                            in1=nfwrep_sb[:], op=OP.mult)
                    gfb = mp.tile([128, H], bf16, name='gfb')
                    nc.vector.tensor_copy(out=gfb[:], in_=gf[:])
                    gfT_ps = mps.tile([128, 128], bf16, space='PSUM',
                                      tag='tr', bufs=1)
                    nc.tensor.transpose(gfT_ps[:], gfb[:], id128_sb[:])
                    gfm = mp.tile([128, GPC], f32, name='gfm')
                    nc.vector.tensor_reduce(
                        out=gfm[:],
                        in_=gfT_ps[:].rearrange('p (g t) -> p g t', t=T),
                        axis=mybir.AxisListType.X, op=OP.add)
                    nc.scalar.activation(out=bgcol_sb[l][:], in_=gfm[:],
                                         func=AF.Identity, scale=1.0 / T,
                                         bias=bcols_sb[:, lcol:lcol + 1])
                    bgb = mp.tile([128, GPC], bf16, name='bgb')
                    nc.vector.tensor_copy(out=bgb[:], in_=bgcol_sb[l][:])
                    grow_ps = mps.tile([GPC, 128], bf16, space='PSUM',
                                       tag='grow')
                    nc.tensor.transpose(grow_ps[:], bgb[:], id128_sb[:])
                    nc.scalar.copy(out=grow_sb[l][:], in_=grow_ps[:])

            # =========== program ===========
            for r_ in range(R):
                xloc = [dpool.tile([NPC, H], bf16, name=f'xloc{l}r{r_}')
                        for l in range(2)]
                xg = [dpool.tile([N, H], bf16, addr_space='Shared',
                                 name=f'xg{l}r{r_}') for l in range(2)]
                ydram = dpool.tile([B, 128], f32, name=f'ydram{r_}')
                yred = dpool.tile([GPC, 128], f32, name=f'yred{r_}')

                do_ag = abl in ('full', 'nostage')
                src1 = xg[0] if do_ag else xf
                src2 = xg[1] if do_ag else xf

                # load this core's own x into xN (natural orientation)
                nc.sync.dma_start(
                    out=xN_sb[:].rearrange('p (t h) -> p t h', h=H),
                    in_=xown[:, :].rearrange('(t p) h -> p t h', p=128))
                nc.vector.memset(st_sb[:], 0.0)
                gin0(xloc)
                if do_ag:
                    nc.gpsimd.collective_compute(
                        'AllGather', mybir.AluOpType.bypass,
                        replica_groups=[list(range(NCORES))],
                        ins=[xloc[0][:]], outs=[xg[0][:]])
                ginAB(1, src1, xloc, None, 0)
                if do_ag:
                    nc.gpsimd.collective_compute(
                        'AllGather', mybir.AluOpType.bypass,
                        replica_groups=[list(range(NCORES))],
                        ins=[xloc[1][:]], outs=[xg[1][:]])
                uid[0] += 1
                with tc.tile_pool(name=f'gsb{r_}', bufs=1) as gsb:
                    ginAB(2, src2, xloc, None, 1)
                    # Y_c = ST^T W3  (partial per-core), then ReduceScatter
                    stb = gsb.tile([128, B], bf16, name='stb')
                    nc.vector.tensor_copy(out=stb[:], in_=st_sb[:])
                    finp_cm = tc.tile_pool(name=f'fin{r_}', bufs=1,
                                           space='PSUM')
                    finp = finp_cm.__enter__()
                    y_ps = finp.tile([B, 128], f32, space='PSUM', tag='yfin')
                    nc.tensor.matmul(out=y_ps[:], lhsT=stb[:],
                                     rhs=wg_sb[3][:], start=True, stop=True)
                    yf = gsb.tile([B, 128], f32, name='yf')
                    nc.scalar.copy(out=yf[:], in_=y_ps[:])
                    nc.sync.dma_start(out=ydram[:, :], in_=yf[:])
                    if abl == 'full':
                        nc.gpsimd.collective_compute(
                            'ReduceScatter', mybir.AluOpType.add,
                            replica_groups=[list(range(NCORES))],
                            ins=[ydram[:]], outs=[yred[:]])
                        yr = gsb.tile([GPC, 128], f32, name='yr')
                        nc.sync.dma_start(out=yr[:], in_=yred[:, :])
                    else:
                        yr = gsb.tile([GPC, 128], f32, name='yr')
                        nc.sync.dma_start(out=yr[:], in_=ydram[0:GPC, :])
                    nc.vector.tensor_tensor(out=yr[:], in0=yr[:],
                                            in1=npgb_sb[:], op=OP.add)
                    nc.sync.dma_start(out=yout[:, :], in_=yr[:])
                    finp_cm.__exit__(None, None, None)

    nc.finalize()
    return nc


def prep_weights(inp, NPG):
    f32 = np.float32
    o = {}
    o['iota'] = np.tile(np.arange(128, dtype=f32)[None, :], (128, 1)).astype(BF)
    o['id128'] = np.eye(128, dtype=f32).astype(BF)
    o['id32'] = np.eye(32, dtype=f32).astype(BF)
    o['ones1'] = np.ones((1, 128), f32).astype(BF)
    selbm = np.zeros((GPC, GPC * 128), f32)
    for g in range(GPC):
        selbm[g, g * 128:(g + 1) * 128] = 1.0
    o['selb'] = selbm.astype(BF)
    wg = np.stack([np.asarray(inp['w_in']), np.asarray(inp['gin_w'])[0],
                   np.asarray(inp['gin_w'])[1], np.asarray(inp['w_out'])])
    o['wg'] = wg.astype(BF)
    o['binrow'] = np.asarray(inp['b_in']).reshape(1, H).astype(BF)
    bcols = np.stack([np.asarray(inp['b_in']), np.asarray(inp['gin_b'])[0],
                      np.asarray(inp['gin_b'])[1],
                      np.asarray(inp['b_out'])], axis=1).astype(f32)
    o['bcols'] = bcols
    o['npgbrep'] = np.tile((NPG * np.asarray(inp['b_out'], f32))[None, :],
                           (GPC, 1)).astype(f32)
    qp = np.zeros((2, 128, 32), f32)
    sb = np.zeros((2, 32, 1), f32)
    scale = 1.0 / np.sqrt(HD)
    for l in range(2):
        qkv_w = np.asarray(inp['qkv_w'])[l]
        qkv_b = np.asarray(inp['qkv_b'])[l]
        q = np.asarray(inp['vt'])[l] @ qkv_w[:, :H] + qkv_b[:H]
        bk = qkv_b[H:2 * H]
        for h in range(NH):
            for t in range(T):
                qp[l, h * HD:(h + 1) * HD, h * T + t] = \
                    q[t, h * HD:(h + 1) * HD] * scale
                sb[l, h * T + t, 0] = \
                    q[t, h * HD:(h + 1) * HD] @ bk[h * HD:(h + 1) * HD] * scale
    # fold Wk into the query projection: scores = (Wk q^T)^T x
    qpf = np.stack([np.asarray(inp['qkv_w'], f32)[l][:, H:2 * H] @ qp[l]
                    for l in range(2)])
    o['qp'] = qpf.astype(BF)
    o['sbias'] = sb
    o['wv'] = np.stack([np.asarray(inp['qkv_w'])[l][:, 2 * H:]
                        for l in range(2)]).astype(BF)
    o['aow'] = np.asarray(inp['ao_w']).astype(BF)
    o['aob2'] = np.stack([
        (np.asarray(inp['qkv_b'])[l][2 * H:] @ np.asarray(inp['ao_w'])[l]
         + np.asarray(inp['ao_b'])[l]).reshape(1, H)
        for l in range(2)]).astype(BF)
    o['inw'] = np.asarray(inp['m_in_w']).astype(BF)
    tmask = (np.arange(128) % T)
    conv_w = np.asarray(inp['m_conv_w'], f32)
    cwm = np.zeros((KC, 128, I), f32)
    for j in range(KC):
        cwm[j] = ((tmask[:, None] >= j).astype(f32)
                  * conv_w[:, KC - 1 - j][None, :])
    o['cwm'] = cwm
    o['cbrep'] = np.tile(np.asarray(inp['m_conv_b'], f32)[None, :], (128, 1))
    xwf = np.asarray(inp['m_x_w'], f32)
    o['xw'] = np.stack([xwf[:128], xwf[128:]]).astype(BF)
    o['dtw'] = np.asarray(inp['m_dt_w']).astype(BF)
    o['dtbrow'] = np.asarray(inp['m_dt_b']).reshape(1, I).astype(BF)
    A = -np.exp(np.asarray(inp['m_A_log'], f32))
    o['arep'] = np.tile(A.reshape(1, I * S), (128, 1)).astype(f32)
    o['drep'] = np.tile(np.asarray(inp['m_D'], f32)[None, :], (128, 1))
    o['nwrep'] = np.tile(np.asarray(inp['m_norm_w'], f32)[None, :], (128, 1))
    o['nfwrep'] = np.tile(np.asarray(inp['m_normf_w'], f32)[None, :],
                          (128, 1))
    shm = np.zeros((128, 5), f32)
    for s in (1, 2, 3, 4):
        shm[:, s] = (tmask >= s).astype(f32)
    o['shm'] = shm
    o['ivm'] = 1.0 - shm
    o['epscol'] = np.full((128, 1), 1e-5, f32)
    outwf = np.asarray(inp['m_out_w'], f32)
    o['outw'] = np.stack([outwf[:128], outwf[128:]]).astype(BF)
    return o


# ---------------- numpy reference (fallback for non-uniform shapes) ---------

def np_reference(inp, N, NPG):
    f32 = np.float32
    x = np.asarray(inp['x'], f32)
    src = np.asarray(inp['edge_index'])[0].astype(np.int64)
    dst = np.asarray(inp['edge_index'])[1].astype(np.int64)
    if 'batch' in inp:
        batch = np.asarray(inp['batch']).astype(np.int64)
    else:
        batch = (np.arange(N) // NPG).astype(np.int64)
    NB = int(inp.get('num_graphs', B))

    def rmsnorm(v, w, eps=1e-5):
        return v / np.sqrt((v * v).mean(-1, keepdims=True) + eps) * w

    def gin(v, W, b):
        agg = np.zeros_like(v)
        np.add.at(agg, dst, v[src])
        return (v + agg) @ np.asarray(W, f32) + np.asarray(b, f32)

    def attn_pool(xb, vt, qkv_w, qkv_b, ao_w, ao_b):
        Bb, Nmax, Hh = xb.shape
        Tt = vt.shape[0]
        q = (vt @ qkv_w[:, :Hh] + qkv_b[:Hh]).reshape(Tt, NH, HD)
        k = (xb @ qkv_w[:, Hh:2 * Hh] + qkv_b[Hh:2 * Hh]).reshape(
            Bb, Nmax, NH, HD)
        v = (xb @ qkv_w[:, 2 * Hh:] + qkv_b[2 * Hh:]).reshape(Bb, Nmax, NH, HD)
        sc = np.einsum('thd,bkhd->bhtk', q, k) / np.sqrt(f32(HD))
        sc = sc - sc.max(-1, keepdims=True)
        e = np.exp(sc)
        at = e / e.sum(-1, keepdims=True)
        ot = np.einsum('bhtk,bkhd->bthd', at, v).reshape(Bb, Tt, Hh)
        return ot @ ao_w + ao_b

    def mamba(tok):
        Bb, Tt, Hh = tok.shape
        res = tok
        hh = rmsnorm(tok, np.asarray(inp['m_norm_w'], f32))
        proj = hh @ np.asarray(inp['m_in_w'], f32)
        u, gate = proj[..., :I], proj[..., I:]
        up = np.pad(u, ((0, 0), (KC - 1, 0), (0, 0)))
        cw = np.asarray(inp['m_conv_w'], f32)
        conv = sum(cw[:, k] * up[:, k:k + Tt, :] for k in range(KC)) \
            + np.asarray(inp['m_conv_b'], f32)
        u = conv * (1 / (1 + np.exp(-conv)))
        ssm = u @ np.asarray(inp['m_x_w'], f32)
        dtr, Bm, Cm = ssm[..., :RK], ssm[..., RK:RK + S], ssm[..., RK + S:]
        dtv = np.logaddexp(0, dtr @ np.asarray(inp['m_dt_w'], f32)
                           + np.asarray(inp['m_dt_b'], f32))
        A = -np.exp(np.asarray(inp['m_A_log'], f32))
        hstate = np.zeros((Bb, I, S), f32)
        ys = []
        for t in range(Tt):
            dA = np.exp(dtv[:, t, :, None] * A)
            dBu = dtv[:, t, :, None] * Bm[:, t, None, :] * u[:, t, :, None]
            hstate = dA * hstate + dBu
            ys.append((hstate * Cm[:, t, None, :]).sum(-1))
        y = np.stack(ys, axis=1) + u * np.asarray(inp['m_D'], f32)
        y = y * (gate * (1 / (1 + np.exp(-gate))))
        out = y @ np.asarray(inp['m_out_w'], f32)
        return rmsnorm(res + out, np.asarray(inp['m_normf_w'], f32))

    starts = np.searchsorted(batch, np.arange(NB))
    pos = np.arange(N) - starts[batch]
    Nmax = NPG
    x = gin(x, inp['w_in'], inp['b_in'])
    for l in range(2):
        g = gin(x, np.asarray(inp['gin_w'])[l], np.asarray(inp['gin_b'])[l])
        xb = np.zeros((NB, Nmax, H), f32)
        xb[batch, pos] = x
        tok = attn_pool(xb, np.asarray(inp['vt'], f32)[l],
                        np.asarray(inp['qkv_w'], f32)[l],
                        np.asarray(inp['qkv_b'], f32)[l],
                        np.asarray(inp['ao_w'], f32)[l],
                        np.asarray(inp['ao_b'], f32)[l])
        gf = mamba(tok)
        x = g + gf.mean(axis=1)[batch]
    x = gin(x, inp['w_out'], inp['b_out'])
    out = np.zeros((NB, H), f32)
    np.add.at(out, batch, x)
    return out


_CACHE = {}
LAST_DEVICE_TIME_NS = 0


def _make_runner(nc, n_cores):
    """Build a reusable jitted SPMD runner (compile once, call many times)."""
    import jax
    from concourse import bass2jax, mybir
    from concourse.bass2jax import _bass_exec_p, install_neuronx_cc_hook, \
        partition_id_tensor

    install_neuronx_cc_hook()
    partition_name = nc.partition_id_tensor.name if nc.partition_id_tensor \
        else None

    in_names, out_names, out_avals, zero_outs = [], [], [], []
    for alloc in nc.m.functions[0].allocations:
        if not isinstance(alloc, mybir.MemoryLocationSet):
            continue
        name = alloc.memorylocations[0].name
        if alloc.kind == 'ExternalInput':
            if name != partition_name:
                in_names.append(name)
        elif alloc.kind == 'ExternalOutput':
            out_names.append(name)
            shape = tuple(alloc.tensor_shape)
            dtype = mybir.dt.np(alloc.dtype)
            out_avals.append(jax.core.ShapedArray(shape, dtype))
            zero_outs.append(np.zeros(shape, dtype))
    n_params = len(in_names)
    n_outs = len(out_avals)
    all_in_names = list(in_names) + list(out_names)
    if partition_name is not None:
        all_in_names.append(partition_name)
    donate = tuple(range(n_params, n_params + n_outs))

    def _body(*args):
        operands = list(args)
        if partition_name is not None:
            operands.append(partition_id_tensor())
        outs = _bass_exec_p.bind(
            *operands,
            out_avals=tuple(out_avals),
            in_names=tuple(all_in_names),
            out_names=tuple(out_names),
            lowering_input_output_aliases=(),
            sim_require_finite=True,
            sim_require_nnan=True,
            nc=nc,
        )
        return tuple(outs)

    devices = jax.devices()[:n_cores]
    mesh = bass2jax.Mesh(np.asarray(devices), ('core',))
    in_specs = (bass2jax.PartitionSpec('core'),) * (n_params + n_outs)
    out_specs = (bass2jax.PartitionSpec('core'),) * n_outs
    sharded = jax.jit(
        bass2jax.shard_map(_body, mesh=mesh, in_specs=in_specs,
                           out_specs=out_specs, check_rep=False),
        donate_argnums=donate, keep_unused=True,
    )

    from jax.sharding import NamedSharding
    shard = NamedSharding(mesh, bass2jax.PartitionSpec('core'))

    def stage(in_maps):
        """device_put all inputs once; returns staged device arrays."""
        import jax
        per_core = [[np.asarray(m[nm]) for nm in in_names] for m in in_maps]
        concat_in = [
            np.concatenate([per_core[c][i] for c in range(n_cores)], axis=0)
            for i in range(n_params)
        ]
        dev_in = [jax.device_put(a, shard) for a in concat_in]
        for a in dev_in:
            a.block_until_ready()
        return dev_in

    def launch(dev_in):
        """one timed launch; returns (outs, wall_ns)."""
        import jax
        concat_zeros = [
            np.zeros((n_cores * z.shape[0], *z.shape[1:]), z.dtype)
            for z in zero_outs
        ]
        dev_zeros = [jax.device_put(a, shard) for a in concat_zeros]
        for a in dev_zeros:
            a.block_until_ready()
        t0 = time.perf_counter_ns()
        out_arrs = sharded(*dev_in, *dev_zeros)
        for o in out_arrs:
            o.block_until_ready()
        wall = time.perf_counter_ns() - t0
        out_arrs = [np.asarray(o) for o in out_arrs]
        outs = [
            {nm: out_arrs[i].reshape(n_cores, *out_avals[i].shape)[c]
             for i, nm in enumerate(out_names)}
            for c in range(n_cores)
        ]
        return outs, wall

    def run(in_maps):
        dev_in = stage(in_maps)
        outs, wall = launch(dev_in)
        run.last_exec_ns = wall
        return outs

    run.last_exec_ns = 0
    run.stage = stage
    run.launch = launch
    return run


def _assemble(outs):
    full = np.zeros((B, H), np.float32)
    for c in range(NCORES):
        full[c * GPC:(c + 1) * GPC] = np.asarray(outs[c]['yout'])
    return full


def _get_compiled(N, NPG, edge_index, R=1):
    """Build (or fetch cached) kernel + edge packing for this graph."""
    ekey = hash(edge_index.tobytes())
    if _CACHE.get('ekey') != (N, NPG, ekey):
        lay, idx_core, dl_core, cnt2_core = prep_edges(edge_index, N, NPG)
        _CACHE.clear()
        _CACHE.update(ekey=(N, NPG, ekey), lay=lay, idx=idx_core, dl=dl_core,
                      cnt2=cnt2_core, runners={})
    lay = _CACHE['lay']
    if R not in _CACHE['runners']:
        nc = build_fused(N, NPG, lay, R=R)
        _CACHE['runners'][R] = _make_runner(nc, NCORES)
    return _CACHE['runners'][R], _CACHE['idx'], _CACHE['dl']


def _device_in_maps(inputs, N, NPG, idx_core, dl_core):
    wmaps = prep_weights(inputs, NPG)
    x_bf = np.asarray(inputs['x'], np.float32).astype(BF)
    cnt2_core = _CACHE['cnt2']
    NPC = N // NCORES
    NPC = N // NCORES
    return [dict(xf=x_bf, xown=x_bf[c * NPC:(c + 1) * NPC],
                 ix16=idx_core[c], dl=dl_core[c],
                 cnt2=cnt2_core[c], **wmaps)
            for c in range(NCORES)]


def kernel(**inputs):
    global LAST_DEVICE_TIME_NS
    x = np.asarray(inputs['x'], np.float32)
    edge_index = np.asarray(inputs['edge_index'], np.int32)
    batch = np.asarray(inputs['batch'], np.int64)
    N = x.shape[0]
    Bn = int(inputs['num_graphs'])
    NPG = int(inputs['nodes_per_graph'])

    uniform = (Bn == B and N == Bn * NPG and NPG % 128 == 0
               and N % (NCORES * 128) == 0 and N % CH == 0
               and np.array_equal(batch, np.arange(N) // NPG))
    if not uniform:
        return np_reference(inputs, N, NPG).astype(np.float32)

    run, idx_core, dl_core = _get_compiled(N, NPG, edge_index, R=1)
    in_maps = _device_in_maps(inputs, N, NPG, idx_core, dl_core)
    outs = run(in_maps)
    LAST_DEVICE_TIME_NS = run.last_exec_ns
    return _assemble(outs).astype(np.float32)
